# revision 13
# baseline (speedup 1.0000x reference)
"""Trainium2 Bass kernel for nn_BackwardCompatibleLoss.

Strategy v4 (reduce-over-local-j, 8 NeuronCores):

Each core owns 512 batch rows (its j-shard of both feat and feat_old).
S-tiles are [i-partitions(128), j-free(512)] so per-row partial sums
Z_i = sum_{j local} exp(100*S - 35) fall out of the ScalarE activation's
accum_out for free.

Per core:
  0. A dummy 256B AllGather fires immediately at kernel entry: it absorbs
     the inter-core start skew AND the ~11us first-collective ncfw ramp
     while local prep runs.
  1. Normalize local fn rows (Square+accum -> Sqrt -> recip -> mul),
     PE-transpose to [d, j]; after EACH 128-row block, stage that quarter
     to DRAM and trigger its AllGather (4 quarters pipelined; the sweep
     consumes quarter q while quarter q+1 is still in flight).
  2. During the gathers: normalize/transpose local fo, compute positive
     logits pos_i = <fn_i, fo_i>, build targets tiles (tgc via contiguous
     load + PE transpose; identities built on-device with affine_select).
  3. Sweep 32 global i-tiles (t = 4r+q): weights = gathered fnT tile
     [d,128i], moving = local foT|fnT [d, 512j] into one [128,1024] PSUM
     pair (n2o | n2n).  Same-label suppression is one fused DVE op
     writing S * (t_j != t_i) to SBUF (masked entries contribute
     exp(-35), ~2e-7 relative on Z).  One Exp per tile writes Z-partials
     into Zall[:, tile] via accum_out.
  4. Finish: AllGather [pos | Z-partials] (18KB), every core then sums
     the 8 partial Z's, adds exp(100*pos-35), takes Ln, subtracts
     100*pos and reduces - each core redundantly produces the full
     scalar loss (no rank-dependent addressing anywhere).
Host reads core 0's scalar / B.  Top-k(1024) in the reference is
replaced by the full masked logsumexp (~1e-5 relative at temp 0.01).
"""

import sys

if "/opt/trn_rl_repo" not in sys.path:
    sys.path.insert(0, "/opt/trn_rl_repo")

import math
from contextlib import ExitStack

import numpy as np

import concourse.bacc as bacc
import concourse.bass as bass
import concourse.tile as tile
from concourse import mybir
from concourse.bass_utils import run_bass_kernel_spmd

F32 = mybir.dt.float32
BF16 = mybir.dt.bfloat16
NP_BF16 = mybir.dt.np(BF16)
AF = mybir.ActivationFunctionType
ALU = mybir.AluOpType

B, D = 4096, 512
NCORES = 8
BL = B // NCORES          # 512 local rows per core
NDB = D // 128            # 4 contraction blocks
NT = B // 128             # 32 global i-tiles
NLB = BL // 128           # 4 local 128-row blocks (= AllGather quarters)
TEMP = 0.01
SCALE = 1.0 / TEMP        # 100
EBIAS = -35.0             # exp(100*S - 35): keeps all exponents in range
RG = [list(range(NCORES))]

_cache = {}


def _build():
    nc = bacc.Bacc("TRN2", target_bir_lowering=False, debug=False,
                   num_devices=NCORES)

    xl = nc.dram_tensor("xl", [BL, D], BF16, kind="ExternalInput")
    yl = nc.dram_tensor("yl", [BL, D], BF16, kind="ExternalInput")
    tl = nc.dram_tensor("tl", [BL], F32, kind="ExternalInput")
    tg = nc.dram_tensor("tg", [B], F32, kind="ExternalInput")
    outp = nc.dram_tensor("outp", [1, 1], F32, kind="ExternalOutput")

    ccd = nc.dram_tensor("ccd", [64], F32)
    ccdo = nc.dram_tensor("ccdo", [NCORES, 64], F32, addr_space="Shared")
    ccin = [nc.dram_tensor(f"ccin{q}", [D, 128], BF16) for q in range(NLB)]
    ccout = [nc.dram_tensor(f"ccout{q}", [NCORES, D, 128], BF16,
                            addr_space="Shared") for q in range(NLB)]
    ccz = nc.dram_tensor("ccz", [BL + B], F32)
    cczo = nc.dram_tensor("cczo", [NCORES, BL + B], F32,
                          addr_space="Shared")

    with ExitStack() as ctx:
        tc = ctx.enter_context(tile.TileContext(nc))
        singles = ctx.enter_context(tc.tile_pool(name="singles", bufs=1))
        work = ctx.enter_context(tc.tile_pool(name="work", bufs=3))
        spool = ctx.enter_context(tc.tile_pool(name="spool", bufs=2))
        mpool = ctx.enter_context(tc.tile_pool(name="mpool", bufs=3))
        psT = ctx.enter_context(tc.tile_pool(name="psT", bufs=2,
                                             space="PSUM"))
        psS = ctx.enter_context(tc.tile_pool(name="psS", bufs=2,
                                             space="PSUM"))
        psO = ctx.enter_context(tc.tile_pool(name="psO", bufs=1,
                                             space="PSUM"))

        # persistent SBUF tensors
        identS = singles.tile([128, 128], BF16, tag="identS")
        identF = singles.tile([128, 128], F32, tag="identF")
        tlb2 = singles.tile([128, 2 * BL], F32, tag="tlb2")
        tgc = singles.tile([128, NT], F32, tag="tgc")
        ones_f = singles.tile([128, 1], F32, tag="ones_f")
        ebias = singles.tile([128, 1], F32, tag="ebias")
        ebias32 = singles.tile([32, 1], F32, tag="ebias32")
        nbF = singles.tile([128, NLB, D], BF16, tag="nbF")
        nbO = singles.tile([128, NLB, D], BF16, tag="nbO")
        fnTl = singles.tile([128, NDB, BL], BF16, tag="fnTl")
        foTl = singles.tile([128, NDB, BL], BF16, tag="foTl")
        gTq = [singles.tile([128, NDB, NCORES * 128], BF16, tag=f"gT{q}",
                            name=f"gT{q}")
               for q in range(NLB)]
        posc = singles.tile([128, NLB], F32, tag="posc")
        posT = singles.tile([4, 128], F32, tag="posT")
        Zall = singles.tile([128, NT], F32, tag="Zall")
        ztS = singles.tile([32, 128], F32, tag="ztS")
        dummy = singles.tile([1, 64], F32, tag="dummy")

        # dummy collective: fires at entry, soaks up skew + ncfw ramp
        nc.vector.memset(dummy, 0.0)
        nc.sync.dma_start(out=ccd.ap(), in_=dummy)
        nc.gpsimd.collective_compute("AllGather", ALU.bypass,
                                     replica_groups=RG,
                                     ins=[ccd.ap().opt()],
                                     outs=[ccdo.ap().opt()])

        # input feature loads - they gate the gathers
        xbs = []
        for blk in range(NLB):
            xb = work.tile([128, D], BF16, tag="xb", name=f"xb{blk}")
            nc.sync.dma_start(out=xb,
                              in_=xl[blk * 128:(blk + 1) * 128, :])
            xbs.append(xb)

        # identities built on-device (cheaper than shipping them 8x)
        onesS = singles.tile([128, 128], BF16, tag="onesS")
        nc.vector.memset(onesS, 1.0)
        nc.gpsimd.affine_select(out=identS, in_=onesS, pattern=[[1, 128]],
                                compare_op=ALU.is_equal, fill=0.0,
                                base=0, channel_multiplier=-1)
        onesF = singles.tile([128, 128], F32, tag="onesF")
        nc.vector.memset(onesF, 1.0)
        nc.gpsimd.affine_select(out=identF, in_=onesF, pattern=[[1, 128]],
                                compare_op=ALU.is_equal, fill=0.0,
                                base=0, channel_multiplier=-1)

        def norm_block(xb, nb, dstT, blk):
            sq = work.tile([128, D], BF16, tag="sq")
            ss = work.tile([128, 1], F32, tag="ss")
            nc.scalar.activation(out=sq, in_=xb, func=AF.Square,
                                 accum_out=ss)
            nrm = work.tile([128, 1], F32, tag="nrm")
            nc.scalar.activation(out=nrm, in_=ss, func=AF.Sqrt)
            rs = work.tile([128, 1], F32, tag="rs")
            nc.vector.reciprocal(rs, nrm)
            nc.vector.tensor_scalar_mul(out=nb[:, blk, :], in0=xb,
                                        scalar1=rs)
            for db in range(NDB):
                pt = psT.tile([128, 128], BF16, tag="pt")
                nc.tensor.transpose(pt, nb[:, blk, db * 128:(db + 1) * 128],
                                    identS)
                nc.vector.tensor_copy(
                    out=dstT[:, db, blk * 128:(blk + 1) * 128], in_=pt)

        # ---- Phase A: per 128-row block: normalize, transpose, gather --
        for q in range(NLB):
            norm_block(xbs[q], nbF, fnTl, q)
            nc.sync.dma_start(
                out=ccin[q].ap().rearrange("(a p) j -> p a j", p=128),
                in_=fnTl[:, :, q * 128:(q + 1) * 128])
            nc.gpsimd.collective_compute("AllGather", ALU.bypass,
                                         replica_groups=RG,
                                         ins=[ccin[q].ap().opt()],
                                         outs=[ccout[q].ap().opt()])

        # ---- Phase B: local fo prep + pos + setup (overlaps gathers) ---
        nc.vector.memset(ebias, EBIAS)
        nc.vector.memset(ebias32, EBIAS)
        nc.vector.memset(ones_f, 1.0)
        for blk in range(NLB):
            yb = work.tile([128, D], BF16, tag="yb")
            nc.sync.dma_start(out=yb,
                              in_=yl[blk * 128:(blk + 1) * 128, :])
            norm_block(yb, nbO, foTl, blk)
            prod = work.tile([128, D], F32, tag="prod")
            nc.vector.tensor_mul(out=prod, in0=nbF[:, blk, :],
                                 in1=nbO[:, blk, :])
            nc.vector.reduce_sum(out=posc[:, blk:blk + 1], in_=prod,
                                 axis=mybir.AxisListType.X)

        # targets: tlb2 broadcast + tgc via contiguous load + PE transpose
        tl_ap = tl.ap()
        tl_b = bass.AP(tensor=tl_ap.tensor, offset=tl_ap.offset,
                       ap=[[0, 128]] + list(tl_ap.ap))
        nc.sync.dma_start(out=tlb2[:, 0:BL], in_=tl_b)
        nc.sync.dma_start(out=tlb2[:, BL:2 * BL], in_=tl_b)
        tgr = singles.tile([32, 128], F32, tag="tgr")
        nc.sync.dma_start(out=tgr, in_=tg.ap().rearrange("(a x) -> a x",
                                                         a=32))
        ptg = psO.tile([128, 128], F32, tag="pscr")
        nc.tensor.transpose(ptg[:, 0:32], tgr, identF[0:32, 0:32])
        nc.vector.tensor_copy(out=tgc, in_=ptg[:, 0:32])
        # pos transposed to [4,128] for the finish
        ppt = psO.tile([128, 128], F32, tag="pscr")
        nc.tensor.transpose(ppt[0:4, :], posc, identF)
        nc.vector.tensor_copy(out=posT, in_=ppt[0:4, :])

        # gathered fnT -> SBUF weights (per quarter)
        for q in range(NLB):
            for r in range(NCORES):
                nc.sync.dma_start(
                    out=gTq[q][:, :, r * 128:(r + 1) * 128],
                    in_=ccout[q][r].rearrange("(a p) j -> p a j", p=128))

        # ---- Phase C: sweep all 32 global i-tiles (t = 4r + q) ---------
        for q in range(NLB):
            for r in range(NCORES):
                t = 4 * r + q
                ps = psS.tile([128, 2 * BL], F32, tag="ps")
                for db in range(NDB):
                    w = gTq[q][:, db, r * 128:(r + 1) * 128]
                    nc.tensor.matmul(ps[:, 0:BL], w, foTl[:, db, :],
                                     start=(db == 0), stop=(db == 3),
                                     skip_group_check=True)
                    nc.tensor.matmul(ps[:, BL:2 * BL], w,
                                     fnTl[:, db, :],
                                     start=(db == 0), stop=(db == 3),
                                     skip_group_check=True)
                sm = mpool.tile([128, 2 * BL], F32, tag="sm")
                nc.vector.scalar_tensor_tensor(
                    out=sm, in0=tlb2, scalar=tgc[:, t:t + 1], in1=ps,
                    op0=ALU.not_equal, op1=ALU.mult)
                scr = spool.tile([128, 2 * BL], BF16, tag="escr")
                nc.scalar.activation(out=scr, in_=sm, func=AF.Exp,
                                     bias=ebias, scale=SCALE,
                                     accum_out=Zall[:, t:t + 1])

        # ---- Phase D: gather [pos | Z], finish redundantly -------------
        pzt = psO.tile([128, 128], F32, tag="pscr")
        nc.tensor.transpose(pzt[0:32, :], Zall, identF)
        nc.vector.tensor_copy(out=ztS, in_=pzt[0:32, :])
        nc.sync.dma_start(out=ccz.ap()[0:BL].rearrange("(a x) -> a x",
                                                       a=4),
                          in_=posT)
        nc.sync.dma_start(out=ccz.ap()[BL:BL + B].rearrange(
            "(a x) -> a x", a=32), in_=ztS)
        nc.gpsimd.collective_compute("AllGather", ALU.bypass,
                                     replica_groups=RG,
                                     ins=[ccz.ap().opt()],
                                     outs=[cczo.ap().opt()])
        Zg = singles.tile([32, NCORES, 128], F32, tag="Zg")
        nc.sync.dma_start(
            out=Zg,
            in_=cczo.ap()[:, BL:BL + B].rearrange("r (a x) -> a r x",
                                                  a=32))
        posg = singles.tile([32, 128], F32, tag="posg")
        for r in range(NCORES):
            nc.sync.dma_start(
                out=posg[4 * r:4 * r + 4, :],
                in_=cczo[r][0:BL].rearrange("(a x) -> a x", a=4))
        s4 = []
        for k in range(4):
            sk = singles.tile([32, 128], F32, tag=f"s4_{k}",
                              name=f"s4_{k}")
            nc.vector.tensor_add(out=sk, in0=Zg[:, 2 * k, :],
                                 in1=Zg[:, 2 * k + 1, :])
            s4.append(sk)
        s2 = []
        for k in range(2):
            sk = singles.tile([32, 128], F32, tag=f"s2_{k}",
                              name=f"s2_{k}")
            nc.vector.tensor_add(out=sk, in0=s4[2 * k], in1=s4[2 * k + 1])
            s2.append(sk)
        Zsum = singles.tile([32, 128], F32, tag="Zsum")
        nc.vector.tensor_add(out=Zsum, in0=s2[0], in1=s2[1])
        posE = singles.tile([32, 128], F32, tag="posE")
        nc.scalar.activation(out=posE, in_=posg, func=AF.Exp,
                             bias=ebias32, scale=SCALE)
        Zfull = singles.tile([32, 128], F32, tag="Zfull")
        nc.vector.tensor_add(out=Zfull, in0=Zsum, in1=posE)
        lnz = singles.tile([32, 128], F32, tag="lnz")
        nc.scalar.activation(out=lnz, in_=Zfull, func=AF.Ln,
                             scale=float(math.exp(-EBIAS)))
        lv = singles.tile([32, 128], F32, tag="lv")
        nc.vector.scalar_tensor_tensor(out=lv, in0=posg, scalar=-SCALE,
                                       in1=lnz, op0=ALU.mult, op1=ALU.add)
        lvs = singles.tile([32, 1], F32, tag="lvs")
        nc.vector.reduce_sum(out=lvs, in_=lv, axis=mybir.AxisListType.X)
        po = psO.tile([128, 128], F32, tag="pscr")
        nc.tensor.matmul(po[0:1, 0:1], ones_f[0:32, :], lvs, start=True,
                         stop=True, skip_group_check=True)
        part = singles.tile([1, 1], F32, tag="part")
        nc.scalar.activation(out=part, in_=po[0:1, 0:1], func=AF.Copy)
        nc.sync.dma_start(out=outp[0:1, 0:1], in_=part)

    nc.compile()
    return nc


def get_nc():
    if "nc" not in _cache:
        _cache["nc"] = _build()
    return _cache["nc"]


def prepare_in_maps(feat, feat_old, targets):
    feat = np.asarray(feat, dtype=np.float32).astype(NP_BF16)
    feat_old = np.asarray(feat_old, dtype=np.float32).astype(NP_BF16)
    tg = np.ascontiguousarray(np.asarray(targets).astype(np.float32))
    in_maps = []
    for c in range(NCORES):
        sl = slice(c * BL, (c + 1) * BL)
        in_maps.append({
            "xl": np.ascontiguousarray(feat[sl]),
            "yl": np.ascontiguousarray(feat_old[sl]),
            "tl": np.ascontiguousarray(tg[sl]),
            "tg": tg,
        })
    return in_maps


def kernel(feat: np.ndarray, feat_old: np.ndarray,
           targets: np.ndarray) -> np.ndarray:
    nc = get_nc()
    in_maps = prepare_in_maps(feat, feat_old, targets)
    res = run_bass_kernel_spmd(nc, in_maps, core_ids=list(range(NCORES)))
    return np.asarray(np.float32(float(res.results[0]["outp"][0, 0]) / B))


if __name__ == "__main__":
    rng = np.random.default_rng(0)
    f = rng.standard_normal((B, D)).astype(np.float32)
    g = rng.standard_normal((B, D)).astype(np.float32)
    t = rng.integers(0, 1000, size=B).astype(np.int64)
    print("loss:", kernel(f, g, t))
